# revision 1
# baseline (speedup 1.0000x reference)
"""DRM attention kernel for 8 Trainium2 NeuronCores.

Sharding: B*H = 32 head-slices; core c handles batch b = c//4 and the 4
heads [4*(c%4), 4*(c%4)+4). Weights replicated (pre-sliced per core on
host). Each core computes its 4 heads' attention output through Wo,
producing a partial [T, DM] for its batch; host sums the 4 partials per
batch (the unshard/reduce step).

All on-device score work happens in transposed layout S^T[j, i] (j = key
pos on partitions, i = query pos on free dim) so the probability tiles
feed the attention*V matmul directly as the stationary operand.

dist(i,j) = |qm_i|^2 + |km_j|^2 - 2 qm.km                  (euclidean)
          + |Uq_i|^2 - 2 sum_r Uq_ir Uk_ijr + sum_r Uk_ijr^2   (low rank)
with Uk_ijr = sum_d U[i,d,r] km[j,d].  The cross term folds into the
linear matmul via w'[i,d] = sum_r U[i,d,r] Uq[i,r]:
  S_lin[j,i] = (-2 km_j).(qm_i + w'_i) + 1*(|qm_i|^2+|Uq_i|^2) + |km_j|^2*1
realized as one K=34 matmul; the quadratic term adds 4 K=32 matmuls
(Uk_r) squared and accumulated per block.  Softmax skips the max
subtraction (all logits <= 0) and gets its denominator from a ones
column appended to V.
"""

import numpy as np

B, T, DM = 2, 512, 1024
H, DH = 16, 64
D, R = 32, 4
TEMP_MIN = 0.5
NCORE = 8
HPC = 4          # heads per core
TC = 4           # 128-chunks along T

_CACHE = {}


def _build():
    import concourse.bass as bass
    import concourse.tile as tile
    from concourse import mybir, bacc

    f32 = mybir.dt.float32
    PSUM = bass.MemorySpace.PSUM
    Alu = mybir.AluOpType
    Act = mybir.ActivationFunctionType
    AxX = mybir.AxisListType.X

    nc = bacc.Bacc("TRN2", target_bir_lowering=False, debug=False)
    f32r = mybir.dt.float32r

    def mm(out, lhsT, rhs, **kw):
        # float32r: same fp32 bits, single-pass PE (4x faster than fp32)
        nc.tensor.matmul(out, lhsT.bitcast(f32r), rhs.bitcast(f32r), **kw)

    xt_d = nc.dram_tensor("xt", [DM, T], f32r, kind="ExternalInput")
    wqk_d = nc.dram_tensor("wqk", [DM, 512], f32r, kind="ExternalInput")
    wv_d = nc.dram_tensor("wv", [DM, 256], f32r, kind="ExternalInput")
    wo_d = nc.dram_tensor("wo", [256, DM], f32r, kind="ExternalInput")
    bqkA_d = nc.dram_tensor("bqkA", [128, 128], f32r, kind="ExternalInput")
    bqkB_d = nc.dram_tensor("bqkB", [128, 128], f32r, kind="ExternalInput")
    wm_d = nc.dram_tensor("wm", [D, 128], f32r, kind="ExternalInput")
    cosr_d = nc.dram_tensor("cosr", [128, T], f32, kind="ExternalInput")
    sinr_d = nc.dram_tensor("sinr", [128, T], f32, kind="ExternalInput")
    maskd_d = nc.dram_tensor("maskd", [128, 128], f32, kind="ExternalInput")
    nit_d = nc.dram_tensor("nit", [128, 1], f32, kind="ExternalInput")
    i4rep_d = nc.dram_tensor("i4rep", [D, 128], f32r, kind="ExternalInput")
    onesrow_d = nc.dram_tensor("onesrow", [1, T], f32r, kind="ExternalInput")
    gsum_d = nc.dram_tensor("gsum", [128, 128], f32r, kind="ExternalInput")
    bsum_d = nc.dram_tensor("bsum", [128, D], f32r, kind="ExternalInput")
    wb1_d = nc.dram_tensor("wb1", [128, 2], f32r, kind="ExternalInput")
    wb2_d = nc.dram_tensor("wb2", [D, 2], f32r, kind="ExternalInput")
    b01_d = nc.dram_tensor("b01", [2, 1], f32, kind="ExternalInput")
    y_d = nc.dram_tensor("y", [T, DM], f32, kind="ExternalOutput")

    with tile.TileContext(nc) as tc:
        with (
            tc.tile_pool(name="const", bufs=1) as cpool,
            tc.tile_pool(name="rope", bufs=4) as rpool,
            tc.tile_pool(name="qkm", bufs=2) as qkmpool,
            tc.tile_pool(name="uu", bufs=2) as uupool,
            tc.tile_pool(name="ext", bufs=2) as extpool,
            tc.tile_pool(name="scr", bufs=6) as scr,
            tc.tile_pool(name="pt", bufs=3) as ptpool,
            tc.tile_pool(name="stk", bufs=2) as stkpool,
            tc.tile_pool(name="psA", bufs=2, space=PSUM) as psA,
            tc.tile_pool(name="psT", bufs=1, space=PSUM) as psT,
            tc.tile_pool(name="psS", bufs=2, space=PSUM) as psS,
            tc.tile_pool(name="psU", bufs=2, space=PSUM) as psU,
            tc.tile_pool(name="psO", bufs=1, space=PSUM) as psO,
        ):
            # ---- constants / weights ----
            xt = [cpool.tile([128, T], f32r, tag=f"xt{k}", name=f"xt{k}") for k in range(8)]
            wqk = [cpool.tile([128, 512], f32r, tag=f"wqk{k}", name=f"wqk{k}") for k in range(8)]
            wv = [cpool.tile([128, 256], f32r, tag=f"wv{k}", name=f"wv{k}") for k in range(8)]
            wo = [cpool.tile([128, DM], f32r, tag=f"wo{p}", name=f"wo{p}") for p in range(2)]
            bqkA = cpool.tile([128, 128], f32r, tag="bqkA")
            bqkB = cpool.tile([128, 128], f32r, tag="bqkB")
            wm = cpool.tile([D, 128], f32r, tag="wm")
            cosr = cpool.tile([128, T], f32, tag="cosr")
            sinr = cpool.tile([128, T], f32, tag="sinr")
            maskd = cpool.tile([128, 128], f32, tag="maskd")
            nit = cpool.tile([128, 1], f32, tag="nit")
            i4rep = cpool.tile([D, 128], f32r, tag="i4rep")
            gsum = cpool.tile([128, 128], f32r, tag="gsum")
            bsum = cpool.tile([128, D], f32r, tag="bsum")
            wb1 = cpool.tile([128, 2], f32r, tag="wb1")
            wb2 = cpool.tile([D, 2], f32r, tag="wb2")
            b01 = cpool.tile([2, 1], f32, tag="b01")
            ones64 = cpool.tile([1, 64], f32r, tag="ones64")
            ones32 = cpool.tile([D, 1], f32r, tag="ones32")
            vext = cpool.tile([128, TC, 260], f32r, tag="vext")

            xt_r = xt_d.ap().rearrange("(k p) t -> k p t", p=128)
            wqk_r = wqk_d.ap().rearrange("(k p) m -> k p m", p=128)
            wv_r = wv_d.ap().rearrange("(k p) m -> k p m", p=128)
            wo_r = wo_d.ap().rearrange("(k p) m -> k p m", p=128)
            for k in range(8):
                nc.sync.dma_start(xt[k][:], xt_r[k])
                nc.sync.dma_start(wqk[k][:], wqk_r[k])
            nc.sync.dma_start(cosr[:], cosr_d.ap())
            nc.sync.dma_start(sinr[:], sinr_d.ap())
            nc.sync.dma_start(bqkA[:], bqkA_d.ap())
            nc.sync.dma_start(bqkB[:], bqkB_d.ap())
            nc.sync.dma_start(wm[:], wm_d.ap())
            nc.sync.dma_start(i4rep[:], i4rep_d.ap())
            nc.sync.dma_start(gsum[:], gsum_d.ap())
            nc.sync.dma_start(bsum[:], bsum_d.ap())
            nc.sync.dma_start(wb1[:], wb1_d.ap())
            nc.sync.dma_start(wb2[:], wb2_d.ap())
            nc.sync.dma_start(b01[:], b01_d.ap())
            nc.sync.dma_start(maskd[:], maskd_d.ap())
            nc.sync.dma_start(nit[:], nit_d.ap())
            for k in range(8):
                nc.sync.dma_start(wv[k][:], wv_r[k])
            for p in range(2):
                nc.sync.dma_start(wo[p][:], wo_r[p])
            nc.gpsimd.memset(ones64[:].bitcast(f32), 1.0)
            nc.gpsimd.memset(ones32[:].bitcast(f32), 1.0)
            nc.gpsimd.memset(vext[:].bitcast(f32), 1.0)

            # ---- QK projection (4 M-chunks) + RoPE + qm/km, per pair ----
            qkm_sig = []     # per pair: [128,T] = [qm_e0; qm_e1; km_e0; km_e1]
            for p in range(2):
                ropeAB = []
                for s in range(2):      # half: A (first 32 dh) / B (second)
                    m = 2 * p + s
                    qk_ps = psA.tile([128, T], f32, tag="psA")
                    for k in range(8):
                        mm(
                            qk_ps[:], wqk[k][:, m * 128:(m + 1) * 128], xt[k][:],
                            start=(k == 0), stop=(k == 7))
                    ropeAB.append(qk_ps)
                A, Bt = ropeAB
                m1 = scr.tile([128, T], f32, tag="scr", bufs=4)
                m2 = scr.tile([128, T], f32, tag="scr", bufs=4)
                nc.vector.tensor_mul(m1[:], A[:], cosr[:])
                nc.vector.tensor_mul(m2[:], Bt[:], sinr[:])
                ropeA = rpool.tile([128, T], f32r, tag="rope")
                nc.vector.tensor_sub(ropeA[:], m1[:], m2[:])
                m3 = scr.tile([128, T], f32, tag="scr", bufs=4)
                m4 = scr.tile([128, T], f32, tag="scr", bufs=4)
                nc.vector.tensor_mul(m3[:], A[:], sinr[:])
                nc.vector.tensor_mul(m4[:], Bt[:], cosr[:])
                ropeB = rpool.tile([128, T], f32r, tag="rope")
                nc.vector.tensor_add(ropeB[:], m3[:], m4[:])

                qkm_ps = psA.tile([128, T], f32, tag="psA")
                mm(qkm_ps[:], bqkA[:], ropeA[:],
                                 start=True, stop=False)
                mm(qkm_ps[:], bqkB[:], ropeB[:],
                                 start=False, stop=True)
                sig = qkmpool.tile([128, T], f32r, tag="qkm")
                nc.scalar.activation(sig[:], qkm_ps[:], Act.Sigmoid)
                qkm_sig.append(sig)

            # ---- V projection into [v_h | 1] blocks of vext ----
            for jc in range(TC):
                v_ps = psA.tile([128, 256], f32, tag="psA")
                for k in range(8):
                    mm(
                        v_ps[:], xt[k][:, jc * 128:(jc + 1) * 128], wv[k][:],
                        start=(k == 0), stop=(k == 7))
                for hl in range(HPC):
                    nc.vector.tensor_copy(vext[:, jc, hl * 65:hl * 65 + 64],
                                           v_ps[:, hl * 64:(hl + 1) * 64])

            # ---- per head ----
            stacked = []
            for p in range(2):
                stk = stkpool.tile([128, T], f32r, tag="stk", name=f"stk{p}")
                stacked.append(stk)

            for hl in range(HPC):
                p, e = hl // 2, hl % 2
                sig = qkm_sig[p]
                # base-partition-0 copies (matmul needs lhsT/rhs aligned)
                qmT_t = scr.tile([D, T], f32r, tag="qmT", bufs=2)
                kmT_t = scr.tile([D, T], f32r, tag="kmT", bufs=2)
                nc.vector.tensor_copy(qmT_t[:], sig[32 * e:32 * e + 32, :])
                nc.vector.tensor_copy(kmT_t[:], sig[64 + 32 * e:96 + 32 * e, :])
                qmT = qmT_t[:]
                kmT = kmT_t[:]

                # U in r-major transposed layout: UUT[(r,d), i], split in
                # two [64,T] tiles so Uk matmul operands align at base 0/32
                uu_sb = []
                for w in range(2):
                    uut_ps = psA.tile([64, T], f32, tag="psA")
                    mm(uut_ps[:], wm[:, w * 64:(w + 1) * 64],
                       qmT, start=True, stop=True)
                    u = uupool.tile([64, T], f32r, tag="uu", name=f"uu{w}", bufs=4)
                    nc.scalar.copy(u[:], uut_ps[:])
                    uu_sb.append(u)

                # km replicated into both 32-partition groups (Uk lhsT)
                kmrep_ps = psA.tile([64, T], f32, tag="psA")
                mm(kmrep_ps[:], i4rep[:, :64], kmT,
                   start=True, stop=True)
                kmrep = scr.tile([64, T], f32r, tag="kmrep", bufs=2)
                nc.scalar.copy(kmrep[:], kmrep_ps[:])

                # Uq / w' / bias entirely in transposed space:
                #   qmrep[(r,d), i] = qmT[d, i]        (i2rep-style matmul)
                #   tmp = UU . qmrep ; UqT_rep = gsum @ tmp   (sum d in block)
                #   tmp2 = UU . UqT_rep ; w'T = bsum @ tmp2   (sum r per d)
                #   biasT = wb1 @ (UqT_rep^2)/1 + wb2 @ qmT^2
                qmrep_ps = psT.tile([128, T], f32, tag="psT")
                mm(qmrep_ps[:], i4rep[:], qmT, start=True, stop=True)
                tmpc = scr.tile([128, T], f32r, tag="tmpc", bufs=2)
                for w in range(2):
                    nc.vector.tensor_mul(tmpc[64 * w:64 * w + 64, :],
                                         uu_sb[w][:],
                                         qmrep_ps[64 * w:64 * w + 64, :])
                uqrep_ps = psT.tile([128, T], f32, tag="psT")
                mm(uqrep_ps[:], gsum[:], tmpc[:], start=True, stop=True)
                tmp2c = scr.tile([128, T], f32r, tag="tmp2c", bufs=2)
                for w in range(2):
                    nc.vector.tensor_mul(tmp2c[64 * w:64 * w + 64, :],
                                         uu_sb[w][:],
                                         uqrep_ps[64 * w:64 * w + 64, :])
                sq2 = scr.tile([128, T], f32r, tag="sq2", bufs=2)
                nc.scalar.square(sq2[:], uqrep_ps[:])
                qmsq = scr.tile([D, T], f32r, tag="qmsq", bufs=2)
                nc.scalar.square(qmsq[:], qmT)

                wpt_ps = psT.tile([D, T], f32, tag="psT")
                mm(wpt_ps[:], bsum[:], tmp2c[:], start=True, stop=True)
                bias_ps = psS.tile([2, T], f32, tag="psS")
                mm(bias_ps[:], wb1[:], sq2[:], start=True, stop=False)
                mm(bias_ps[:], wb2[:], qmsq[:], start=False, stop=True)

                gt = extpool.tile([34, T], f32r, tag="gt")
                nc.vector.tensor_add(gt[:32, :], wpt_ps[:], qmT)
                nc.scalar.activation(gt[32:34, :], bias_ps[:], Act.Identity,
                                     bias=b01[:], scale=1.0)

                # lhsT of the linear matmul: kmT_ext [34, T]
                # rows: 0:32 = -2*km, 32 = |km|^2 (pairs gt's ones row),
                # 33 = ones (pairs gt's bias row)
                kme = extpool.tile([34, T], f32r, tag="kme")
                nc.vector.tensor_scalar_mul(kme[:32, :], kmT, -2.0)
                kmsq = scr.tile([D, T], f32r, tag="kmsq", bufs=2)
                nc.scalar.square(kmsq[:], kmT)
                k2m_ps = psS.tile([1, T], f32, tag="psS")
                mm(k2m_ps[:], ones32[:], kmsq[:],
                                 start=True, stop=True)
                nc.scalar.copy(kme[32:33, :], k2m_ps[:])
                nc.sync.dma_start(kme[33:34, :], onesrow_d.ap())

                # score blocks, exp, attn*v
                ot_ps = psO.tile([128, T], f32, tag="psO")
                for jc in range(TC):
                    ioff = 128 * jc
                    ni = T - ioff
                    s_ps = psS.tile([128, T], f32, tag="psS")
                    mm(
                        s_ps[:, :ni],
                        kme[:, jc * 128:(jc + 1) * 128],
                        gt[:, ioff:],
                        start=True, stop=True)
                    tsq = []
                    uk_pools = [psU, psU, psT, psA]
                    for r in range(R):
                        uk_ps = uk_pools[r].tile([128, T], f32,
                                                 tag=uk_pools[r].name,
                                                 name=f"uk{r}")
                        w, rr = r // 2, r % 2
                        mm(
                            uk_ps[:, :ni],
                            kmrep[rr * 32:(rr + 1) * 32,
                                  jc * 128:(jc + 1) * 128],
                            uu_sb[w][rr * 32:(rr + 1) * 32, ioff:],
                            start=True, stop=True)
                        t = scr.tile([128, T], f32, tag="tsq", bufs=6)
                        nc.scalar.square(t[:, :ni], uk_ps[:, :ni])
                        tsq.append(t)
                    a01 = scr.tile([128, T], f32, tag="acc", bufs=4)
                    a23 = scr.tile([128, T], f32, tag="acc", bufs=4)
                    nc.gpsimd.tensor_add(a01[:, :ni], tsq[0][:, :ni], tsq[1][:, :ni])
                    nc.gpsimd.tensor_add(a23[:, :ni], tsq[2][:, :ni], tsq[3][:, :ni])
                    a03 = scr.tile([128, T], f32, tag="acc", bufs=4)
                    nc.vector.tensor_add(a03[:, :ni], a01[:, :ni], a23[:, :ni])
                    ssb = scr.tile([128, T], f32, tag="ssb", bufs=3)
                    nc.vector.tensor_add(ssb[:, :ni], a03[:, :ni], s_ps[:, :ni])

                    pt = ptpool.tile([128, T], f32r, tag="pt")
                    nc.scalar.activation(pt[:, :ni], ssb[:, :ni], Act.Exp,
                                         scale=nit[:, 0:1])
                    nc.vector.tensor_mul(pt[:, :128], pt[:, :128], maskd[:])
                    mm(
                        ot_ps[:65, ioff:],
                        vext[:, jc, hl * 65:(hl + 1) * 65],
                        pt[:, :ni],
                        start=(jc == 0), stop=(jc == 3),
                        skip_group_check=True)

                # normalize and stack into [o_e0; o_e1] per pair: broadcast
                # the raw denominator row across 64 partitions via a K=1
                # matmul (operand bases must match: ones64[64:65] aligns with
                # the denominator at PSUM partition 64), then reciprocal at
                # base 0 (reciprocal_approx misreads nonzero-base PSUM).
                den_sb = scr.tile([1, T], f32r, tag="den", bufs=2)
                nc.scalar.copy(den_sb[:], ot_ps[64:65, :])
                bc_ps = psS.tile([64, T], f32, tag="psS")
                mm(bc_ps[:], ones64[:], den_sb[:], start=True, stop=True)
                bc_sb = scr.tile([64, T], f32, tag="bcsb", bufs=2)
                nc.vector.reciprocal_approx_fast(out=bc_sb[:], in_=bc_ps[:])
                nc.vector.tensor_mul(stacked[p][64 * e:64 * e + 64, :],
                                     ot_ps[:64, :], bc_sb[:])

            # ---- output projection (partial y for this core's 4 heads) ----
            for ic in range(TC):
                for ncn in range(2):
                    y_ps = psS.tile([128, 512], f32, tag="psS", name="y_ps")
                    mm(
                        y_ps[:], stacked[0][:, ic * 128:(ic + 1) * 128],
                        wo[0][:, ncn * 512:(ncn + 1) * 512],
                        start=True, stop=False)
                    mm(
                        y_ps[:], stacked[1][:, ic * 128:(ic + 1) * 128],
                        wo[1][:, ncn * 512:(ncn + 1) * 512],
                        start=False, stop=True)
                    y_sb = scr.tile([128, 512], f32, tag="ysb", bufs=2)
                    nc.scalar.copy(y_sb[:], y_ps[:])
                    nc.sync.dma_start(
                        y_d.ap()[ic * 128:(ic + 1) * 128,
                                 ncn * 512:(ncn + 1) * 512],
                        y_sb[:])

    nc.compile()
    return nc


def _r32(a):
    """Round fp32 to fp32r (11-bit mantissa, RNE) so on-device fp32r matmuls
    see pre-rounded operands."""
    u = np.ascontiguousarray(a, np.float32).view(np.uint32).astype(np.uint64)
    u = (u + 0x7FF + ((u >> 12) & 1)) & 0xFFFFF000
    return u.astype(np.uint32).view(np.float32)


def _rope_tables():
    inv_freq = 1.0 / (10000.0 ** (np.arange(0, DH, 2, dtype=np.float32) / DH))
    t = np.arange(T, dtype=np.float32)
    freqs = t[:, None] * inv_freq[None, :]          # [T, 32]
    return np.cos(freqs), np.sin(freqs)


def _prep_inputs(x, Wq, Wk, Wv, Wo, Wqm, Wkm, Wmetric, temperature):
    x = np.asarray(x, np.float32)
    Wq, Wk, Wv, Wo = (np.asarray(w, np.float32) for w in (Wq, Wk, Wv, Wo))
    Wqm, Wkm = np.asarray(Wqm, np.float32), np.asarray(Wkm, np.float32)
    Wmetric = np.asarray(Wmetric, np.float32)
    temp = float(np.asarray(temperature))

    cosf, sinf = _rope_tables()
    cosr = np.ascontiguousarray(np.tile(cosf.T, (4, 1)))   # [128, T]
    sinr = np.ascontiguousarray(np.tile(sinf.T, (4, 1)))

    bqkA = np.zeros((128, 128), np.float32)
    bqkB = np.zeros((128, 128), np.float32)
    for ee in range(2):
        bqkA[64 * ee:64 * ee + 32, 32 * ee:32 * ee + 32] = Wqm[0:32]
        bqkA[64 * ee + 32:64 * ee + 64, 64 + 32 * ee:96 + 32 * ee] = Wkm[0:32]
        bqkB[64 * ee:64 * ee + 32, 32 * ee:32 * ee + 32] = Wqm[32:64]
        bqkB[64 * ee + 32:64 * ee + 64, 64 + 32 * ee:96 + 32 * ee] = Wkm[32:64]

    wm = np.ascontiguousarray(
        Wmetric.reshape(D, D, R).transpose(0, 2, 1).reshape(D, D * R))

    jj, ii = np.meshgrid(np.arange(128), np.arange(128), indexing="ij")
    maskd = (jj <= ii).astype(np.float32)
    nit = np.full((128, 1), -1.0 / max(temp, TEMP_MIN), np.float32)
    i4rep = np.ascontiguousarray(np.tile(np.eye(D, dtype=np.float32), (1, 4)))
    rr, dd = np.arange(128) // D if False else np.arange(128) // 32, np.arange(128) % 32
    gsum = np.zeros((128, 128), np.float32)   # [(r',d'), (r,d)] = [r'==r]
    for a in range(128):
        for bcol in range(128):
            if a // 32 == bcol // 32:
                gsum[a, bcol] = 1.0
    bsum = np.zeros((128, D), np.float32)     # [(r,d), d'] = [d==d']
    for a in range(128):
        bsum[a, a % 32] = 1.0
    wb1 = np.zeros((128, 2), np.float32); wb1[:, 1] = 1.0 / 32.0
    wb2 = np.zeros((D, 2), np.float32); wb2[:, 1] = 1.0
    b01 = np.array([[1.0], [0.0]], np.float32)

    in_maps = []
    for c in range(NCORE):
        b, g = c // 4, c % 4
        lh0 = 4 * g
        wqk = np.empty((DM, 512), np.float32)
        for p in range(2):
            for s in range(2):
                m = 2 * p + s
                for ee in range(2):
                    h = lh0 + 2 * p + ee
                    cq = Wq[:, h * 64 + 32 * s: h * 64 + 32 * s + 32]
                    ck = Wk[:, h * 64 + 32 * s: h * 64 + 32 * s + 32]
                    wqk[:, m * 128 + 64 * ee: m * 128 + 64 * ee + 32] = cq
                    wqk[:, m * 128 + 64 * ee + 32: m * 128 + 64 * ee + 64] = ck
        in_maps.append({
            "xt": _r32(x[b].T),
            "wqk": _r32(wqk),
            "wv": _r32(Wv[:, lh0 * 64: lh0 * 64 + 256]),
            "wo": _r32(Wo[lh0 * 64: lh0 * 64 + 256, :]),
            "bqkA": _r32(bqkA),
            "bqkB": _r32(bqkB),
            "wm": _r32(wm),
            "cosr": cosr,
            "sinr": sinr,
            "maskd": maskd,
            "nit": nit,
            "i4rep": i4rep,
            "gsum": gsum, "bsum": bsum, "wb1": wb1, "wb2": wb2, "b01": b01,
            "onesrow": np.ones((1, T), np.float32),
        })
    return in_maps


def kernel(x, Wq, Wk, Wv, Wo, Wqm, Wkm, Wmetric, temperature, **_):
    from concourse import bass_utils

    if "nc" not in _CACHE:
        _CACHE["nc"] = _build()
    nc = _CACHE["nc"]

    in_maps = _prep_inputs(x, Wq, Wk, Wv, Wo, Wqm, Wkm, Wmetric, temperature)
    res = bass_utils.run_bass_kernel_spmd(nc, in_maps,
                                          core_ids=list(range(NCORE)))
    y = np.zeros((B, T, DM), np.float32)
    for b in range(B):
        acc = res.results[4 * b]["y"].astype(np.float32)
        for g in range(1, 4):
            acc = acc + res.results[4 * b + g]["y"]
        y[b] = acc
    return y



# revision 8
# speedup vs baseline: 1.1165x; 1.1165x over previous
"""DRM attention kernel for 8 Trainium2 NeuronCores.

Sharding: B*H = 32 head-slices; core c handles batch b = c//4 and the 4
heads [4*(c%4), 4*(c%4)+4). Weights replicated (pre-sliced per core on
host). Each core computes its 4 heads' attention output through Wo,
producing a partial [T, DM] for its batch; host sums the 4 partials per
batch.

Score work in transposed layout S^T[j, i] (j = key pos on partitions,
i = query pos on free dim).

Math (per head):
  dist(i,j) = |qm_i-km_j|^2 + sum_r (U_i^T(qm_i-km_j))_r^2
Softmax over j is invariant to any additive term that depends only on
i, so the |qm_i|^2 + |Uq_i|^2 part of the expansion is DROPPED.  What
remains:
  S[j,i] = -2 km_j.(qm_i + w'_i)          (K=32 matmul, w' = U_i Uq_i)
         + |km_j|^2                       (per-j: folded into exp bias)
         + sum_r Uk_ijr^2                 (4 K=32 matmuls, squared)
  p = exp(-(S)/t);  denominator via ones column in V.

Tensor-engine discipline: all K=32 matmuls run in the (32,128) tiling
mode with explicit tile_position so 4 of them occupy the four 32-row
strips of the PE concurrently (uu/kmrep spans, uk spans), and mode
switches (which drain the PE) only happen at a handful of phase
boundaries.
"""

import numpy as np

B, T, DM = 2, 512, 1024
H, DH = 16, 64
D, R = 32, 4
TEMP_MIN = 0.5
NCORE = 8
HPC = 4          # heads per core
TC = 4           # 128-chunks along T (key blocks)

# score units per head: (jc, i0, ni), one per key block jc
UNITS = [(_jc, 128 * _jc, T - 128 * _jc) for _jc in range(TC)]
NU = len(UNITS)   # 4

_CACHE = {}


def _build():
    import concourse.bass as bass
    import concourse.tile as tile
    from concourse import mybir, bacc

    f32 = mybir.dt.float32
    PSUM = bass.MemorySpace.PSUM
    Alu = mybir.AluOpType
    Act = mybir.ActivationFunctionType

    nc = bacc.Bacc("TRN2", target_bir_lowering=False, debug=False)
    f32r = mybir.dt.float32r

    def mm(out, lhsT, rhs, **kw):
        nc.tensor.matmul(out, lhsT.bitcast(f32r), rhs.bitcast(f32r), **kw)

    xt_d = nc.dram_tensor("xt", [DM, T], f32r, kind="ExternalInput")
    wqk_d = nc.dram_tensor("wqk", [DM, 512], f32r, kind="ExternalInput")
    wv_d = nc.dram_tensor("wv", [DM, 256], f32r, kind="ExternalInput")
    wo_d = nc.dram_tensor("wo", [256, DM], f32r, kind="ExternalInput")
    bqkA_d = nc.dram_tensor("bqkA", [128, 128], f32r, kind="ExternalInput")
    bqkB_d = nc.dram_tensor("bqkB", [128, 128], f32r, kind="ExternalInput")
    wm4_d = nc.dram_tensor("wm4", [128, 128], f32r, kind="ExternalInput")
    i4rep4_d = nc.dram_tensor("i4rep4", [128, 128], f32r, kind="ExternalInput")
    cosr_d = nc.dram_tensor("cosr", [128, T], f32, kind="ExternalInput")
    sinr_d = nc.dram_tensor("sinr", [128, T], f32, kind="ExternalInput")
    maskd_d = nc.dram_tensor("maskd", [128, 128], f32, kind="ExternalInput")
    nit_d = nc.dram_tensor("nit", [128, 1], f32, kind="ExternalInput")
    negit_d = nc.dram_tensor("negit", [128, 2], f32r, kind="ExternalInput")
    gsum_d = nc.dram_tensor("gsum", [128, 128], f32r, kind="ExternalInput")
    bsum_d = nc.dram_tensor("bsum", [128, D], f32r, kind="ExternalInput")
    y_d = nc.dram_tensor("y", [T, DM], f32, kind="ExternalOutput")

    with tile.TileContext(nc) as tc:
        with (
            tc.tile_pool(name="const", bufs=1) as cpool,
            tc.tile_pool(name="rope", bufs=4) as rpool,
            tc.tile_pool(name="qkm", bufs=2) as qkmpool,
            tc.tile_pool(name="uu", bufs=4) as uupool,
            tc.tile_pool(name="kmr", bufs=4) as kmrpool,
            tc.tile_pool(name="hb", bufs=1) as hbpool,     # per-head [128,T] persistents
            tc.tile_pool(name="scr", bufs=6) as scr,
            tc.tile_pool(name="sq", bufs=4) as sqpool,
            tc.tile_pool(name="pt", bufs=10) as ptpool,
            tc.tile_pool(name="ob", bufs=1) as obpool,
            tc.tile_pool(name="psUK", bufs=1, space=PSUM) as psUK,
            tc.tile_pool(name="psC", bufs=2, space=PSUM) as psC,
            tc.tile_pool(name="psD", bufs=2, space=PSUM) as psD,
        ):
            # ---- constants / weights ----
            xt = [cpool.tile([128, T], f32r, tag=f"xt{k}", name=f"xt{k}") for k in range(8)]
            wqk = [cpool.tile([128, 512], f32r, tag=f"wqk{k}", name=f"wqk{k}") for k in range(8)]
            wv = [cpool.tile([128, 256], f32r, tag=f"wv{k}", name=f"wv{k}") for k in range(8)]
            wo = [cpool.tile([128, DM], f32r, tag=f"wo{p}", name=f"wo{p}") for p in range(2)]
            bqkA = cpool.tile([128, 128], f32r, tag="bqkA")
            bqkB = cpool.tile([128, 128], f32r, tag="bqkB")
            wm4 = cpool.tile([128, 128], f32r, tag="wm4")
            i4rep4 = cpool.tile([128, 128], f32r, tag="i4rep4")
            cosr = cpool.tile([128, T], f32, tag="cosr")
            sinr = cpool.tile([128, T], f32, tag="sinr")
            maskd = cpool.tile([128, 128], f32, tag="maskd")
            nit = cpool.tile([128, 1], f32, tag="nit")
            negit = cpool.tile([128, 2], f32r, tag="negit")
            gsum = cpool.tile([128, 128], f32r, tag="gsum")
            bsum = cpool.tile([128, D], f32r, tag="bsum")
            ones64 = cpool.tile([1, 64], f32r, tag="ones64")
            vext = cpool.tile([128, TC, 260], f32r, tag="vext")

            xt_r = xt_d.ap().rearrange("(k p) t -> k p t", p=128)
            wqk_r = wqk_d.ap().rearrange("(k p) m -> k p m", p=128)
            wv_r = wv_d.ap().rearrange("(k p) m -> k p m", p=128)
            wo_r = wo_d.ap().rearrange("(k p) m -> k p m", p=128)
            for k in range(8):
                nc.sync.dma_start(xt[k][:], xt_r[k])
                nc.sync.dma_start(wqk[k][:], wqk_r[k])
            nc.sync.dma_start(cosr[:], cosr_d.ap())
            nc.sync.dma_start(sinr[:], sinr_d.ap())
            nc.sync.dma_start(bqkA[:], bqkA_d.ap())
            nc.sync.dma_start(bqkB[:], bqkB_d.ap())
            nc.sync.dma_start(wm4[:], wm4_d.ap())
            nc.sync.dma_start(i4rep4[:], i4rep4_d.ap())
            nc.sync.dma_start(gsum[:], gsum_d.ap())
            nc.sync.dma_start(bsum[:], bsum_d.ap())
            nc.sync.dma_start(maskd[:], maskd_d.ap())
            nc.sync.dma_start(nit[:], nit_d.ap())
            nc.sync.dma_start(negit[:], negit_d.ap())
            for k in range(8):
                nc.sync.dma_start(wv[k][:], wv_r[k])
            for p in range(2):
                nc.sync.dma_start(wo[p][:], wo_r[p])
            nc.gpsimd.memset(ones64[:].bitcast(f32), 1.0)
            nc.gpsimd.memset(vext[:].bitcast(f32), 1.0)

            # ================= phase 1: (128,128) projections =================
            # QK projection + RoPE + qm/km sigmoid, per pair
            qkm_sig = []     # per pair: [128,T] = [qm_e0; qm_e1; km_e0; km_e1]
            for p in range(2):
                ropeAB = []
                for s in range(2):      # dh half: A (first 32) / B (second)
                    m = 2 * p + s
                    qk_ps = psC.tile([128, T], f32, tag="psC", name="qk_ps")
                    for k in range(8):
                        mm(qk_ps[:], wqk[k][:, m * 128:(m + 1) * 128], xt[k][:],
                           start=(k == 0), stop=(k == 7))
                    ropeAB.append(qk_ps)
                A, Bt = ropeAB
                m1 = scr.tile([128, T], f32, tag="scr", bufs=4)
                m2 = scr.tile([128, T], f32, tag="scr", bufs=4)
                nc.vector.tensor_mul(m1[:], A[:], cosr[:])
                nc.vector.tensor_mul(m2[:], Bt[:], sinr[:])
                ropeA = rpool.tile([128, T], f32r, tag="rope")
                nc.vector.tensor_sub(ropeA[:], m1[:], m2[:])
                m3 = scr.tile([128, T], f32, tag="scr", bufs=4)
                m4 = scr.tile([128, T], f32, tag="scr", bufs=4)
                nc.vector.tensor_mul(m3[:], A[:], sinr[:])
                nc.vector.tensor_mul(m4[:], Bt[:], cosr[:])
                ropeB = rpool.tile([128, T], f32r, tag="rope")
                nc.vector.tensor_add(ropeB[:], m3[:], m4[:])

                qkm_ps = psD.tile([128, T], f32, tag="psD", name="qkm_ps")
                mm(qkm_ps[:], bqkA[:], ropeA[:], start=True, stop=False)
                mm(qkm_ps[:], bqkB[:], ropeB[:], start=False, stop=True)
                sig = qkmpool.tile([128, T], f32r, tag="qkm")
                nc.scalar.activation(sig[:], qkm_ps[:], Act.Sigmoid)
                qkm_sig.append(sig)

            # V projection into [v_h | 1] blocks of vext
            for jc in range(TC):
                v_ps = psC.tile([128, 256], f32, tag="psC", name="v_ps")
                for k in range(8):
                    mm(v_ps[:], xt[k][:, jc * 128:(jc + 1) * 128], wv[k][:],
                       start=(k == 0), stop=(k == 7))
                for hl in range(HPC):
                    nc.vector.tensor_copy(vext[:, jc, hl * 65:hl * 65 + 64],
                                          v_ps[:, hl * 64:(hl + 1) * 64])

            # ============ phase 2: (32,128) spans: uu/kmrep/qmrep ============
            # per pair: span A = {uu_e0@s0, uu_e1@s1, kmrep_e0@s2, kmrep_e1@s3}
            #           span B = {qmrep_e0@s0, qmrep_e1@s1}
            # wm4 rows 0:64 = wm at strips 0,1 ; rows 64:128 = i4rep at 2,3
            uu_sb = [None] * HPC      # [128, T] f32r, rows (32r+d') = U_r
            kmr_sb = [None] * HPC     # [128, T] f32r, km replicated x4
            tmpc_l = [None] * HPC
            for p in range(2):
                sig = qkm_sig[p]
                # quad: seg 0/1 = uu_e0/uu_e1, seg 2/3 = kmr_e0/kmr_e1
                quad = psUK.tile([128, 4, T], f32, tag="quad", name=f"quad{p}")
                for e in range(2):
                    mm(quad[:, e, :], wm4[32 * e:32 * e + 32, :],
                       sig[32 * e:32 * e + 32, :],
                       start=True, stop=True, tile_position=(32 * e, 0),
                       skip_group_check=True)
                for e in range(2):
                    mm(quad[:, 2 + e, :], i4rep4[64 + 32 * e:96 + 32 * e, :],
                       sig[64 + 32 * e:96 + 32 * e, :],
                       start=True, stop=True, tile_position=(64 + 32 * e, 0),
                       skip_group_check=True)
                for e in range(2):
                    h = 2 * p + e
                    u = uupool.tile([128, T], f32r, tag="uu", name=f"uu{h}")
                    nc.vector.tensor_copy(u[:], quad[:, e, :])
                    uu_sb[h] = u
                    kr = kmrpool.tile([128, T], f32r, tag="kmr", name=f"kmr{h}")
                    nc.vector.tensor_copy(kr[:], quad[:, 2 + e, :])
                    kmr_sb[h] = kr
                for e in range(2):
                    h = 2 * p + e
                    pool = psC if e == 0 else psD
                    qm_ps = pool.tile([128, T], f32, tag=pool.name, name=f"qm_ps{h}")
                    mm(qm_ps[:], i4rep4[32 * e:32 * e + 32, :],
                       sig[32 * e:32 * e + 32, :],
                       start=True, stop=True, tile_position=(32 * e, 0))
                    t = scr.tile([128, T], f32r, tag="tmpc", bufs=4)
                    nc.vector.scalar_tensor_tensor(
                        t[:], qm_ps[:], 1.0, uu_sb[h][:],
                        op0=Alu.bypass, op1=Alu.mult)
                    tmpc_l[h] = t

            # ============ phase 3: (128,128) gsum -> uqrep ============
            tmp2_l = []
            for h in range(HPC):
                pool = psC if h % 2 == 0 else psD
                uq_ps = pool.tile([128, T], f32, tag=pool.name, name=f"uq_ps{h}")
                mm(uq_ps[:], gsum[:], tmpc_l[h][:], start=True, stop=True)
                t2 = scr.tile([128, T], f32r, tag="tmp2", bufs=4)
                nc.vector.scalar_tensor_tensor(
                    t2[:], uq_ps[:], 1.0, uu_sb[h][:],
                    op0=Alu.bypass, op1=Alu.mult)
                tmp2_l.append(t2)

            # ============ phase 4: (128,32) bsum -> w' ============
            gt4 = hbpool.tile([128, T], f32r, tag="gt4")
            kme4 = hbpool.tile([128, T], f32r, tag="kme4")
            kmsq4 = hbpool.tile([128, T], f32r, tag="kmsq4")
            for h in range(HPC):
                p, e = h // 2, h % 2
                sig = qkm_sig[p]
                wpt_ps = psC.tile([32, T], f32, tag="psC", name=f"wpt_ps{h}")
                mm(wpt_ps[:], bsum[:], tmp2_l[h][:], start=True, stop=True)
                nc.vector.scalar_tensor_tensor(
                    gt4[32 * h:32 * h + 32, :], wpt_ps[:],
                    1.0, sig[32 * e:32 * e + 32, :],
                    op0=Alu.bypass, op1=Alu.add)
                nc.vector.tensor_scalar_mul(
                    kme4[32 * h:32 * h + 32, :],
                    sig[64 + 32 * e:96 + 32 * e, :], -2.0)
                nc.gpsimd.tensor_mul(
                    kmsq4[32 * h:32 * h + 32, :],
                    sig[64 + 32 * e:96 + 32 * e, :],
                    sig[64 + 32 * e:96 + 32 * e, :])

            # ============ phase 5: (32,128) k2col: -|km|^2/t ============
            # one [128, 16] psum tile; col 4h+jc = bias for (head h, block jc)
            k2q = psUK.tile([128, 4, T], f32, tag="quad", name="k2q")
            for h in range(HPC):
                for jc in range(TC):
                    mm(k2q[:, h, 2 * jc:2 * jc + 2],
                       kmsq4[32 * h:32 * h + 32, jc * 128:(jc + 1) * 128],
                       negit[32 * h:32 * h + 32, :],
                       start=(jc == 0), stop=True, tile_position=(32 * h, 0),
                       skip_group_check=True)
            k2sb = hbpool.tile([128, 4, 8], f32, tag="k2sb")
            nc.vector.tensor_copy(k2sb[:], k2q[:, :, 0:8])

            # ============ phase 6: score loop ============
            # per (head, unit): (32,128)-mode spans {uk x4 strips, lin@strip h}
            # then elementwise chain, then (128,128) attnV interleaved.
            ot_ps_l = [None] * HPC
            pt_tiles = [[None] * NU for _ in range(HPC)]

            def score_unit(h, u):
                jc, i0, ni = UNITS[u]
                uk = psUK.tile([128, 4, T], f32, tag="quad", name="uk")
                s_ps = psC.tile([128, 512], f32, tag="psC", name="s_ps")
                for r in range(R):
                    mm(uk[:, r, :ni],
                       kmr_sb[h][32 * r:32 * r + 32, jc * 128:(jc + 1) * 128],
                       uu_sb[h][32 * r:32 * r + 32, i0:i0 + ni],
                       start=True, stop=True,
                       tile_position=(32 * r, 0),
                       skip_group_check=True)
                mm(s_ps[:, :ni],
                   kme4[32 * h:32 * h + 32, jc * 128:(jc + 1) * 128],
                   gt4[32 * h:32 * h + 32, i0:i0 + ni],
                   start=True, stop=True, tile_position=(32 * h, 0))
                # elementwise: one big square (ACT) ; adds (gpsimd/DVE)
                sq4 = sqpool.tile([128, 4, 512], f32, tag="sq4", bufs=2)
                nc.scalar.square(sq4[:, :, :ni], uk[:, :, :ni])
                dd = scr.tile([128, 2, 512], f32, tag="dd", bufs=2)
                nc.gpsimd.tensor_add(dd[:, :, :ni], sq4[:, 0:2, :ni],
                                     sq4[:, 2:4, :ni])
                cc = scr.tile([128, 512], f32, tag="cc", bufs=2)
                nc.vector.tensor_add(cc[:, :ni], dd[:, 0, :ni], dd[:, 1, :ni])
                ssb = scr.tile([128, 512], f32, tag="ssb", bufs=2)
                nc.vector.scalar_tensor_tensor(
                    ssb[:, :ni], s_ps[:, :ni], 1.0, cc[:, :ni],
                    op0=Alu.bypass, op1=Alu.add)
                pt = ptpool.tile([128, 512], f32r, tag="pt")
                nc.scalar.activation(pt[:, :ni], ssb[:, :ni], Act.Exp,
                                     bias=k2sb[:, h, 2 * jc:2 * jc + 1],
                                     scale=nit[:, 0:1])
                nc.gpsimd.tensor_mul(pt[:, :128], pt[:, :128], maskd[:])
                pt_tiles[h][u] = pt

            def attn_v(h):
                ot_ps = psD.tile([128, T], f32, tag="psD", name=f"ot{h}")
                ot_ps_l[h] = ot_ps
                for u in range(NU):
                    jc, i0, ni = UNITS[u]
                    mm(ot_ps[:65, i0:i0 + ni],
                       vext[:, jc, h * 65:(h + 1) * 65],
                       pt_tiles[h][u][:, :ni],
                       start=(jc == 0), stop=(u == NU - 1),
                       skip_group_check=True)
                    pt_tiles[h][u] = None

            num_sb = [None] * HPC
            den_sb = [None] * HPC

            def drain_head(h):
                ns = obpool.tile([64, T], f32r, tag=f"num{h}", name=f"num{h}")
                nc.vector.tensor_copy(ns[:], ot_ps_l[h][:64, :])
                ds = obpool.tile([1, T], f32r, tag=f"den{h}", name=f"den{h}")
                nc.scalar.copy(ds[:], ot_ps_l[h][64:65, :])
                num_sb[h], den_sb[h] = ns, ds

            for u in range(NU):
                score_unit(0, u)
            for u in range(NU):
                score_unit(1, u)
            attn_v(0)
            for u in range(NU):
                score_unit(2, u)
            attn_v(1)
            drain_head(0)
            for u in range(NU):
                score_unit(3, u)
            attn_v(2)
            drain_head(1)
            attn_v(3)
            drain_head(2)
            drain_head(3)

            # ============ phase 7: (32,64) denominator broadcast ============
            bc_ps_l = []
            for h in range(HPC):
                bc_ps = psC.tile([64, T], f32, tag="psC", name=f"bc{h}")
                mm(bc_ps[:], ones64[:], den_sb[h][:], start=True, stop=True)
                bc_ps_l.append(bc_ps)
            stacked = []
            for p in range(2):
                stk = hbpool.tile([128, T], f32r, tag=f"stk{p}", name=f"stk{p}")
                stacked.append(stk)
            for h in range(HPC):
                p, e = h // 2, h % 2
                rec = scr.tile([64, T], f32, tag="rec", bufs=2)
                nc.vector.reciprocal_approx_fast(out=rec[:], in_=bc_ps_l[h][:])
                nc.vector.tensor_mul(stacked[p][64 * e:64 * e + 64, :],
                                     num_sb[h][:], rec[:])

            # ============ phase 8: (128,128) output projection ============
            for ic in range(TC):
                for ncn in range(2):
                    y_ps = psC.tile([128, 512], f32, tag="psC", name="y_ps")
                    mm(y_ps[:], stacked[0][:, ic * 128:(ic + 1) * 128],
                       wo[0][:, ncn * 512:(ncn + 1) * 512],
                       start=True, stop=False)
                    mm(y_ps[:], stacked[1][:, ic * 128:(ic + 1) * 128],
                       wo[1][:, ncn * 512:(ncn + 1) * 512],
                       start=False, stop=True)
                    y_sb = scr.tile([128, 512], f32, tag="ysb", bufs=2)
                    nc.scalar.copy(y_sb[:], y_ps[:])
                    nc.sync.dma_start(
                        y_d.ap()[ic * 128:(ic + 1) * 128,
                                 ncn * 512:(ncn + 1) * 512],
                        y_sb[:])

    nc.compile()
    return nc


def _r32(a):
    """Round fp32 to fp32r (11-bit mantissa, RNE)."""
    u = np.ascontiguousarray(a, np.float32).view(np.uint32).astype(np.uint64)
    u = (u + 0x7FF + ((u >> 12) & 1)) & 0xFFFFF000
    return u.astype(np.uint32).view(np.float32)


def _rope_tables():
    inv_freq = 1.0 / (10000.0 ** (np.arange(0, DH, 2, dtype=np.float32) / DH))
    t = np.arange(T, dtype=np.float32)
    freqs = t[:, None] * inv_freq[None, :]          # [T, 32]
    return np.cos(freqs), np.sin(freqs)


def _prep_inputs(x, Wq, Wk, Wv, Wo, Wqm, Wkm, Wmetric, temperature):
    x = np.asarray(x, np.float32)
    Wq, Wk, Wv, Wo = (np.asarray(w, np.float32) for w in (Wq, Wk, Wv, Wo))
    Wqm, Wkm = np.asarray(Wqm, np.float32), np.asarray(Wkm, np.float32)
    Wmetric = np.asarray(Wmetric, np.float32)
    temp = float(np.asarray(temperature))

    cosf, sinf = _rope_tables()
    cosr = np.ascontiguousarray(np.tile(cosf.T, (4, 1)))   # [128, T]
    sinr = np.ascontiguousarray(np.tile(sinf.T, (4, 1)))

    bqkA = np.zeros((128, 128), np.float32)
    bqkB = np.zeros((128, 128), np.float32)
    for ee in range(2):
        bqkA[64 * ee:64 * ee + 32, 32 * ee:32 * ee + 32] = Wqm[0:32]
        bqkA[64 * ee + 32:64 * ee + 64, 64 + 32 * ee:96 + 32 * ee] = Wkm[0:32]
        bqkB[64 * ee:64 * ee + 32, 32 * ee:32 * ee + 32] = Wqm[32:64]
        bqkB[64 * ee + 32:64 * ee + 64, 64 + 32 * ee:96 + 32 * ee] = Wkm[32:64]

    wm = np.ascontiguousarray(
        Wmetric.reshape(D, D, R).transpose(0, 2, 1).reshape(D, D * R))
    i4rep = np.ascontiguousarray(np.tile(np.eye(D, dtype=np.float32), (1, 4)))
    wm4 = np.zeros((128, 128), np.float32)
    wm4[0:32] = wm
    wm4[32:64] = wm
    wm4[64:96] = i4rep
    wm4[96:128] = i4rep
    i4rep4 = np.zeros((128, 128), np.float32)
    for s in range(4):
        i4rep4[32 * s:32 * s + 32] = i4rep

    jj, ii = np.meshgrid(np.arange(128), np.arange(128), indexing="ij")
    maskd = (jj <= ii).astype(np.float32)
    it = -1.0 / max(temp, TEMP_MIN)
    nit = np.full((128, 1), it, np.float32)
    negit = np.full((128, 2), it, np.float32)
    gsum = np.zeros((128, 128), np.float32)   # [(r',d'), (r,d)] = [r'==r]
    for a in range(128):
        for bcol in range(128):
            if a // 32 == bcol // 32:
                gsum[a, bcol] = 1.0
    bsum = np.zeros((128, D), np.float32)     # [(r,d), d'] = [d==d']
    for a in range(128):
        bsum[a, a % 32] = 1.0

    in_maps = []
    for c in range(NCORE):
        b, g = c // 4, c % 4
        lh0 = 4 * g
        wqk = np.empty((DM, 512), np.float32)
        for p in range(2):
            for s in range(2):
                m = 2 * p + s
                for ee in range(2):
                    h = lh0 + 2 * p + ee
                    cq = Wq[:, h * 64 + 32 * s: h * 64 + 32 * s + 32]
                    ck = Wk[:, h * 64 + 32 * s: h * 64 + 32 * s + 32]
                    wqk[:, m * 128 + 64 * ee: m * 128 + 64 * ee + 32] = cq
                    wqk[:, m * 128 + 64 * ee + 32: m * 128 + 64 * ee + 64] = ck
        in_maps.append({
            "xt": _r32(x[b].T),
            "wqk": _r32(wqk),
            "wv": _r32(Wv[:, lh0 * 64: lh0 * 64 + 256]),
            "wo": _r32(Wo[lh0 * 64: lh0 * 64 + 256, :]),
            "bqkA": _r32(bqkA),
            "bqkB": _r32(bqkB),
            "wm4": _r32(wm4),
            "i4rep4": i4rep4,
            "cosr": cosr,
            "sinr": sinr,
            "maskd": maskd,
            "nit": nit,
            "negit": negit,
            "gsum": gsum,
            "bsum": bsum,
        })
    return in_maps


def kernel(x, Wq, Wk, Wv, Wo, Wqm, Wkm, Wmetric, temperature, **_):
    from concourse import bass_utils

    if "nc" not in _CACHE:
        _CACHE["nc"] = _build()
    nc = _CACHE["nc"]

    in_maps = _prep_inputs(x, Wq, Wk, Wv, Wo, Wqm, Wkm, Wmetric, temperature)
    res = bass_utils.run_bass_kernel_spmd(nc, in_maps,
                                          core_ids=list(range(NCORE)))
    y = np.zeros((B, T, DM), np.float32)
    for b in range(B):
        acc = res.results[4 * b]["y"].astype(np.float32)
        for g in range(1, 4):
            acc = acc + res.results[4 * b + g]["y"]
        y[b] = acc
    return y
